# revision 13
# baseline (speedup 1.0000x reference)
"""Trainium2 Bass kernel for nn_DVQuantumLinear (v2).

Math: the reference's 4-qubit circuit (CNOTs only couple qubits 0-1) makes
the Z-expectations *linear* in the 13-dim trig basis
    F(x) = [1, cos x0, sin x0, cos x1, sin x1,
            cos(x0+x1), sin(x0+x1), cos(x0-x1), sin(x0-x1),
            cos x2, sin x2, cos x3, sin x3]
and out[b,:] = F(x_b) @ C for a (13,2) matrix C fit host-side (exact).

Device pipeline per core (65536 samples + 1024 pad), data parallel x8:
  - angles shipped pre-wrapped to [-pi, pi) as 12 rows/sample
    (rows 2t / 2t+1 = theta_t + pi/2 / theta_t), partition p = g*12 + r
    over G=10 groups; 6656 cols per group.
  - column range [0, A): int8 angles -> ScalarE Sin activation (scale pi/128)
  - column range [A, 6656): int16 angles -> DVE deg-5 odd-poly sin in fp16
    (conv, square, Horner via tensor_scalar/tensor_tensor)
  - PE: one fp16 matmul per 512-col chunk, W [120, 20] block-diag in
    (group, output) -> PSUM stripes, 6 chunks packed per PSUM bank
  - Pool/DVE: PSUM -> SBUF fp16 downconvert copies per bank
  - 3 output DMAs (fp16); host unpermutes stripes, adds the constant term,
    and casts to f32.
A tiny early matmul warms the PE p-state clock so real matmuls run at full
rate. Constant feature folded into host postprocessing.
"""

import numpy as np

N_CORES = 8
B_TOTAL = 524288
BC = B_TOTAL // N_CORES      # 65536 real samples per core
G = 10                       # sample groups stacked on partitions
BG = 6656                    # padded samples per group
BCP = G * BG                 # 66560 padded samples per core
RPG = 12                     # feature rows per group
ROWS = G * RPG               # 120
POLY_LO = 3072               # int16 / DVE poly region [POLY_LO, POLY_HI)
POLY_HI = 4096
POLY_COLS = POLY_HI - POLY_LO
NCH = 13                     # 512-col matmul chunks
MM_N = 512
# ACT op column splits (first small for early start; last micro for tail)
ACT_SPLITS = [(0, 768), (768, 2048), (2048, 3072), (4096, 6144),
              (6144, 6656)]
# DVE poly chunk splits
POLY_SPLITS = [3072, 3584, 4096]
# deg-5 odd minimax-ish poly for sin on [-pi, pi]: sin(t) ~ t*(A1 + A3 t^2 + A5 t^4)
# fit on the int16 grid in _poly_coeffs()

_PROGRAM_CACHE = {}


def _poly_coeffs():
    g = np.arange(-32768, 32768) * (np.pi / 32768.0)
    t = g * g
    A = np.stack([g, g * t, g * t * t], axis=1)
    coef, *_ = np.linalg.lstsq(A, np.sin(g), rcond=None)
    return coef  # [a1, a3, a5]


def _fit_coeffs(weights, head_w, head_b):
    """Exact linear coefficients C (13, 2) with out = F(x) @ C."""
    w = np.asarray(weights, np.float64)
    hw = np.asarray(head_w, np.float64)
    hb = np.asarray(head_b, np.float64)

    rng = np.random.default_rng(1234)
    x = rng.normal(size=(2048, 4))

    bsz = x.shape[0]
    state = np.zeros((bsz,) + (2,) * 4, dtype=np.complex128)
    state[:, 0, 0, 0, 0] = 1.0

    def apply_batched(st, gates, wire):
        st = np.moveaxis(st, 1 + wire, -1)
        st = np.einsum("b...a,bca->b...c", st, gates)
        return np.moveaxis(st, -1, 1 + wire)

    def apply_shared(st, gate, wire):
        st = np.moveaxis(st, 1 + wire, -1)
        st = np.einsum("...a,ca->...c", st, gate)
        return np.moveaxis(st, -1, 1 + wire)

    for i in range(4):
        c, s = np.cos(x[:, i] / 2), np.sin(x[:, i] / 2)
        gset = np.zeros((bsz, 2, 2), np.complex128)
        gset[:, 0, 0] = c
        gset[:, 0, 1] = -s
        gset[:, 1, 0] = s
        gset[:, 1, 1] = c
        state = apply_batched(state, gset, i)
    for _rep in range(2):
        for i in range(4):
            e = np.exp(-0.5j * w[0, i, 0])
            rz = np.array([[e, 0], [0, np.conj(e)]], np.complex128)
            state = apply_shared(state, rz, i)
            c, s = np.cos(w[0, i, 1] / 2), np.sin(w[0, i, 1] / 2)
            ry = np.array([[c, -s], [s, c]], np.complex128)
            state = apply_shared(state, ry, i)
        state = np.concatenate(
            [state[:, :1], np.flip(state[:, 1:], axis=2)], axis=1
        )
    probs = (state * np.conj(state)).real
    zexp = []
    for i in range(4):
        axes = tuple(a for a in range(1, 5) if a != 1 + i)
        marg = probs.sum(axis=axes)
        zexp.append(marg[:, 0] - marg[:, 1])
    z = np.stack(zexp, -1)

    c, s = np.cos(x), np.sin(x)
    S = x[:, 0] + x[:, 1]
    D = x[:, 0] - x[:, 1]
    F = np.stack(
        [np.ones(bsz), c[:, 0], s[:, 0], c[:, 1], s[:, 1],
         np.cos(S), np.sin(S), np.cos(D), np.sin(D),
         c[:, 2], s[:, 2], c[:, 3], s[:, 3]],
        axis=-1,
    )
    Wfit, _, _, _ = np.linalg.lstsq(F, z, rcond=None)
    resid = np.abs(F @ Wfit - z).max()
    assert resid < 1e-9, f"feature basis fit failed: resid={resid}"
    C = Wfit @ hw.T                       # (13, 2)
    C[0, :] += hb
    return C


def _build_program():
    import concourse.bacc as bacc
    import concourse.bass as bass
    import concourse.mybir as mybir
    import concourse.tile as tile

    f32 = mybir.dt.float32
    f16 = mybir.dt.float16
    i8 = mybir.dt.int8
    i16 = mybir.dt.int16
    nc = bacc.Bacc("TRN2", target_bir_lowering=False, debug=False,
                   num_devices=N_CORES)

    a8_d = nc.dram_tensor("a8", [ROWS, BG - POLY_COLS], i8,
                          kind="ExternalInput").ap()
    a16_d = nc.dram_tensor("a16", [ROWS, POLY_COLS], i16,
                           kind="ExternalInput").ap()
    w0_d = nc.dram_tensor("wmat0", [ROWS, 128], f16,
                          kind="ExternalInput").ap()
    wr_d = nc.dram_tensor("wmatr", [ROWS, 5 * 128], f16,
                          kind="ExternalInput").ap()
    y_d = nc.dram_tensor("yraw", [ROWS, 3 * MM_N], f16,
                         kind="ExternalOutput").ap()

    a1, a3, a5 = (float(v) for v in _poly_coeffs())

    with tile.TileContext(nc) as tc:
        with (
            tc.tile_pool(name="const", bufs=1) as cpool,
            tc.tile_pool(name="io", bufs=1) as iopool,
            tc.tile_pool(name="psum", bufs=1, space=bass.MemorySpace.PSUM) as opool,
        ):
            # sustained PE warmup: ~3us of dummy matmuls keeps the PE
            # p-state ramp clock running so real matmuls visit at full rate
            zt = cpool.tile([128, 512], f16)
            nc.vector.memset(zt[:, 0:8], 0)
            wp = opool.tile([8, MM_N], f32, tag="warm")
            for _ in range(7):
                nc.tensor.matmul(wp[:, :], zt[:, 0:8], zt[:, :],
                                 start=True, stop=True)

            x8 = iopool.tile([ROWS, BG - POLY_COLS], i8)
            x16 = iopool.tile([ROWS, POLY_COLS], i16)
            w0_sb = cpool.tile([ROWS, 128], f16)
            wr_sb = cpool.tile([ROWS, 5 * 128], f16)

            # x8 col c maps to phi col c for c<3072, else c+POLY_COLS
            # w slice 0 early via HWDGE; the rest via SWDGE (Pool queue)
            nc.gpsimd.dma_start(wr_sb[:], wr_d[:])
            nc.sync.dma_start(x8[:, 0:768], a8_d[:, 0:768])
            nc.sync.dma_start(w0_sb[:], w0_d[:])
            nc.sync.dma_start(x8[:, 768:2048], a8_d[:, 768:2048])
            nc.sync.dma_start(x16[:, 0:512], a16_d[:, 0:512])
            nc.sync.dma_start(x8[:, 2048:3584], a8_d[:, 2048:3584])
            nc.sync.dma_start(x16[:, 512:1024], a16_d[:, 512:1024])
            nc.sync.dma_start(x8[:, 3584:5632], a8_d[:, 3584:5632])

            phi = iopool.tile([ROWS, BG], f16)
            th = iopool.tile([ROWS, POLY_COLS], f16)
            tsq = iopool.tile([ROWS, POLY_COLS], f16)
            pacc = iopool.tile([ROWS, POLY_COLS], f16)

            sin = mybir.ActivationFunctionType.Sin
            mult = mybir.AluOpType.mult
            addop = mybir.AluOpType.add

            # issue poly (DVE) ops per chunk
            for lo, hi in zip(POLY_SPLITS[:-1], POLY_SPLITS[1:]):
                l = lo - POLY_LO
                h = hi - POLY_LO
                nc.vector.tensor_scalar(
                    th[:, l:h], x16[:, l:h], float(np.pi / 32768.0), None, mult)
                nc.vector.tensor_tensor(tsq[:, l:h], th[:, l:h], th[:, l:h], mult)
                nc.vector.tensor_scalar(
                    pacc[:, l:h], tsq[:, l:h], a5, a3, mult, addop)
                nc.vector.tensor_tensor(
                    pacc[:, l:h], pacc[:, l:h], tsq[:, l:h], mult)
                nc.vector.tensor_scalar(pacc[:, l:h], pacc[:, l:h], 1.0, a1,
                                        mult, addop)
                nc.vector.tensor_tensor(
                    phi[:, lo:hi], pacc[:, l:h], th[:, l:h], mult)

            # ACT sin ops (x8 is phi minus the poly window)
            for lo, hi in ACT_SPLITS:
                xl = lo if lo < POLY_LO else lo - POLY_COLS
                xh = xl + (hi - lo)
                nc.scalar.activation(
                    phi[:, lo:hi], x8[:, xl:xh], sin,
                    scale=float(np.pi / 128.0))

            # matmuls: chunk t -> bank (t//6), rows 20*(t%6)
            bank0 = opool.tile([128, MM_N], f32, tag="bank0")
            bank1 = opool.tile([128, MM_N], f32, tag="bank1")
            bank2 = opool.tile([128, MM_N], f32, tag="bank2")
            banks = [bank0, bank1, bank2]
            y_sb = iopool.tile([ROWS, 3 * MM_N], f16)
            for t in range(12):
                b, m = divmod(t, 6)
                wsl = w0_sb[:, :] if m == 0 else wr_sb[:, 128 * (m - 1):128 * m]
                nc.tensor.matmul(
                    banks[b][:, :], wsl,
                    phi[:, MM_N * t:MM_N * (t + 1)],
                    start=(m == 0), stop=(m == 5))
            nc.tensor.matmul(bank2[:, :], w0_sb[:, :],
                             phi[:, 6144:6656], start=True, stop=True)

            # PSUM -> SBUF fp16 copies spread over DVE + ACT, then DMAs
            copyf = mybir.ActivationFunctionType.Copy
            nc.vector.tensor_copy(y_sb[:, 0:512], bank0[0:ROWS, :])
            nc.gpsimd.dma_start(y_d[:, 0:512], y_sb[:, 0:512])
            nc.vector.tensor_copy(y_sb[0:20, 1024:1536], bank2[0:20, :])
            nc.sync.dma_start(y_d[0:20, 1024:1536], y_sb[0:20, 1024:1536])
            nc.scalar.activation(y_sb[:, 512:1024], bank1[0:ROWS, :], copyf)
            nc.sync.dma_start(y_d[:, 512:1024], y_sb[:, 512:1024])
    nc.compile()
    return nc


def _host_tensors(weights, head_w, head_b):
    C = _fit_coeffs(weights, head_w, head_b)  # (13, 2) f64
    rowcoef = C[1:, :].astype(np.float16)     # (12, 2)
    const = C[0, :].astype(np.float32)        # (2,)
    wmat = np.zeros((ROWS, 6 * 128), np.float16)
    for m in range(6):
        for g in range(G):
            for j in range(2):
                wmat[g * RPG:(g + 1) * RPG,
                     128 * m + 20 * m + j * G + g] = rowcoef[:, j]
    return wmat[:, :128], np.ascontiguousarray(wmat[:, 128:]), const


def _host_angles(x):
    """(12, B) pre-reduced pre-biased angle rows, float64 in [-pi, pi)."""
    xt = np.asarray(x, np.float32).T          # (4, B)
    theta = np.empty((6, x.shape[0]), np.float64)
    theta[0] = xt[0]
    theta[1] = xt[1]
    theta[2] = xt[0].astype(np.float64) + xt[1]
    theta[3] = xt[0].astype(np.float64) - xt[1]
    theta[4] = xt[2]
    theta[5] = xt[3]
    two_pi = 2 * np.pi
    ang = np.empty((RPG, x.shape[0]), np.float64)
    ang[0::2] = (theta + np.pi / 2 + np.pi) % two_pi - np.pi
    ang[1::2] = (theta + np.pi) % two_pi - np.pi
    return ang


def kernel(x, weights, head_w, head_b):
    from concourse.bass_utils import run_bass_kernel_spmd

    x = np.asarray(x, np.float32)
    assert x.shape == (B_TOTAL, 4)
    w0, wr, const = _host_tensors(weights, head_w, head_b)
    ang = _host_angles(x)                     # (12, B) f64

    nc = _PROGRAM_CACHE.get("nc")
    if nc is None:
        nc = _build_program()
        _PROGRAM_CACHE["nc"] = nc

    in_maps = []
    for c in range(N_CORES):
        blk = np.zeros((RPG, BCP), np.float64)
        blk[:, :BC] = ang[:, c * BC:(c + 1) * BC]
        # rows (g, r) -> partition g*12+r; col within group
        grp = blk.reshape(RPG, G, BG).transpose(1, 0, 2).reshape(ROWS, BG)
        act_part = np.concatenate(
            [grp[:, :POLY_LO], grp[:, POLY_HI:]], axis=1)
        a8 = np.clip(np.round(act_part * (128.0 / np.pi)),
                     -128, 127).astype(np.int8)
        a16 = np.clip(np.round(grp[:, POLY_LO:POLY_HI] * (32768.0 / np.pi)),
                      -32768, 32767).astype(np.int16)
        in_maps.append({
            "a8": np.ascontiguousarray(a8),
            "a16": np.ascontiguousarray(a16),
            "wmat0": w0, "wmatr": wr,
        })

    res = run_bass_kernel_spmd(nc, in_maps, core_ids=list(range(N_CORES)))
    y = np.empty((B_TOTAL, 2), np.float32)
    for c in range(N_CORES):
        yr = res.results[c]["yraw"].astype(np.float32)   # (120, 1536)
        yc = np.empty((G, BG, 2), np.float32)
        for t in range(12):
            b, m = divmod(t, 6)
            blk = yr[20 * m:20 * m + 20, 512 * b:512 * (b + 1)]
            # rows 20*m + j*10 + g
            yc[:, MM_N * t:MM_N * (t + 1), :] = (
                blk.reshape(2, G, MM_N).transpose(1, 2, 0)
            )
        blk = yr[0:20, 1024:1536]
        yc[:, 6144:6656, :] = blk.reshape(2, G, MM_N).transpose(1, 2, 0)
        yc += const[None, None, :]
        y[c * BC:(c + 1) * BC, :] = yc.reshape(G * BG, 2)[:BC]
    return y
